# revision 86
# baseline (speedup 1.0000x reference)
"""LinOSS layer Trainium2 kernel (pipelined, merged r/i layout).

Math (same derivation as before): per-state recurrence collapses to
rank-2 modulated prefix sums
    u_t = s * Bu_t
    E = cumsum(T1 * u);  F = cumsum(T2 * u)
    x_t = sin(t th) * (E_t + oE) + cos(t th) * (F_t + oF)
    T1 = gamma*cos + sin;  T2 = cos - gamma*sin
with oE/oF the fold-chunk carry offsets.

Layout: 128 partitions = 2 fold-chunks x (32 real + 32 imag states);
free dim = 4096 time cols.  r/i share theta so one table row set serves
both; E/F merge the four scans of the old layout into two.

Pipeline: host sends input pre-transposed plus all four tables (no
on-chip table build, no DMA transpose).  Per 1024-col chunk: Bu matmuls
-> PSUM, DVE stt modulate straight from PSUM (accum_out collects row
sums for the carry), chained DVE scans (AP initial).  Carry offsets are
ready after the last modulate, so demod (ACT bias-add, DVE/Pool mults)
and projection matmuls overlap the remaining scans.
"""

import numpy as np

L, H, P = 8192, 128, 256
NCORES = 8
SLOC = P // NCORES          # 32 states per core
RI = 2 * SLOC               # 64 r+i rows per fold chunk
FOLD = 2
CL = L // FOLD              # 4096 free cols
SC = 1024                   # modulate/scan chunk
K = CL // SC                # 4
JT = 512                    # demod/project chunk
NJ = CL // JT               # 8

_CACHE: dict = {}


def _build_bass(split_waits=True):
    import concourse.bass as bass
    import concourse.mybir as mybir
    import concourse.tile as tile

    dt = mybir.dt.float32
    bt = mybir.dt.bfloat16
    Alu = mybir.AluOpType
    Ident = mybir.ActivationFunctionType.Identity

    nc = bass.Bass(
        trn_type="TRN2",
        target_bir_lowering=False,
        debug=False,
        num_devices=NCORES,
    )

    inpT_d = nc.dram_tensor("inpT", [H, L], bt, kind="ExternalInput").ap()
    Bt_d = nc.dram_tensor("Bt", [H, RI], bt, kind="ExternalInput").ap()
    Cx_d = nc.dram_tensor("Cx", [128, H], bt, kind="ExternalInput").ap()
    dD_d = nc.dram_tensor("dD", [H, H], bt, kind="ExternalInput").ap()
    Wm_d = nc.dram_tensor("Wm", [128, 128], dt, kind="ExternalInput").ap()
    T1_d = nc.dram_tensor("T1", [128, CL], bt, kind="ExternalInput").ap()
    T2_d = nc.dram_tensor("T2", [128, CL], bt, kind="ExternalInput").ap()
    sinT_d = nc.dram_tensor("sinT", [128, CL], bt, kind="ExternalInput").ap()
    cosT_d = nc.dram_tensor("cosT", [128, CL], bt, kind="ExternalInput").ap()
    outp = nc.dram_tensor("outp", [H, L], bt, kind="ExternalOutput").ap()

    with tile.TileContext(nc) as tc:
        cpool = tc.alloc_tile_pool(name="const", bufs=1)
        big = tc.alloc_tile_pool(name="big", bufs=1)
        work = tc.alloc_tile_pool(name="work", bufs=3)
        opool = tc.alloc_tile_pool(name="opool", bufs=8)
        pbu = tc.alloc_tile_pool(name="pbu", bufs=2, space="PSUM")
        pout = tc.alloc_tile_pool(name="pout", bufs=3, space="PSUM")
        poff = tc.alloc_tile_pool(name="poff", bufs=1, space="PSUM")

        Bt = cpool.tile([H, RI], bt)
        Cx = cpool.tile([128, H], bt)
        dD = cpool.tile([H, H], bt)
        Wm = cpool.tile([128, 128], dt)
        inpT = big.tile([H, L], bt, tag="inpT")
        T1 = big.tile([128, CL], bt, tag="T1")
        T2 = big.tile([128, CL], bt, tag="T2")
        sinT = big.tile([128, CL], bt, tag="sinT")
        cosT = big.tile([128, CL], bt, tag="cosT")
        E = big.tile([128, CL], bt, tag="E")
        F = big.tile([128, CL], bt, tag="F")

        NACC = K + 1            # k=0 runs as 2 sub-chunks
        ones_b = cpool.tile([128, SC], bt)
        zinit = cpool.tile([128, 1], dt)
        acc1 = cpool.tile([128, NACC], dt)
        acc2 = cpool.tile([128, NACC], dt)
        fins = cpool.tile([128, 2], dt)
        offs = cpool.tile([128, 2], dt)

        nc.gpsimd.memset(zinit[:], 0.0)
        nc.gpsimd.memset(ones_b[:], 1.0)

        # warm the lazy ACT function table before any scalar-queue DMA
        # issues so the first real ACT op isn't delayed by the 1.3us load
        wdum = cpool.tile([128, 1], dt)
        nc.scalar.activation(wdum[:], zinit[:], Ident)

        # -- input / table DMAs --
        # One DMA's descriptors drain through a single HW queue (~25 GB/s =
        # 5.1us per 512-col piece), and each issue costs ~0.6us of its
        # sequencer, which also dispatches that engine's compute ops.  So:
        # budget few early issues per queue, strict need order, and issue
        # the rest from inside the phase-1 loop.
        def pc512(q, dst, src, lo):
            q.dma_start(out=dst[:, lo : lo + 512], in_=src[:, lo : lo + 512])

        nc.sync.dma_start(out=Bt[:], in_=Bt_d)
        pc512(nc.sync, inpT, inpT_d, 0)
        pc512(nc.scalar, inpT, inpT_d, CL)
        pc512(nc.sync, T1, T1_d, 0)
        pc512(nc.scalar, T2, T2_d, 0)
        pc512(nc.sync, inpT, inpT_d, 512)
        pc512(nc.scalar, inpT, inpT_d, CL + 512)
        pc512(nc.sync, T1, T1_d, 512)
        pc512(nc.scalar, T2, T2_d, 512)
        # k=1 pieces behind the k=0 criticals, balanced sync/scalar
        pc512(nc.scalar, inpT, inpT_d, SC)
        pc512(nc.sync, inpT, inpT_d, CL + SC)
        pc512(nc.scalar, T1, T1_d, SC)
        pc512(nc.sync, T2, T2_d, SC)
        pc512(nc.scalar, inpT, inpT_d, SC + 512)
        pc512(nc.sync, inpT, inpT_d, CL + SC + 512)
        pc512(nc.scalar, T1, T1_d, SC + 512)
        pc512(nc.sync, T2, T2_d, SC + 512)
        # k=2/3 upfront on gpsimd (~2 parallel SWDGE streams; 15-25us of
        # slack before use keeps the cold-run window closed)
        for k in range(2, K):
            for s in range(2):
                lo = k * SC + s * 512
                for c in range(FOLD):
                    pc512(nc.gpsimd, inpT, inpT_d, c * CL + lo)
                pc512(nc.gpsimd, T1, T1_d, lo)
                pc512(nc.gpsimd, T2, T2_d, lo)
        nc.sync.dma_start(out=Wm[:], in_=Wm_d)
        nc.sync.dma_start(out=Cx[:], in_=Cx_d)
        nc.sync.dma_start(out=dD[:], in_=dD_d)
        # sin/cos (phase 2 only) on gpsimd BEHIND the k2/k3 pieces: its ~2
        # serial SWDGE streams naturally delay these transfers past the
        # phase-1 contention window, and they land ~35-50us with plenty of
        # slack before their ~48us+ first use
        for k in range(K):
            for s in range(2):
                lo = k * SC + s * 512
                pc512(nc.gpsimd, sinT, sinT_d, lo)
                pc512(nc.gpsimd, cosT, cosT_d, lo)

        # -- phase 1: Bu matmuls -> modulate -> chained scans --
        # k=0 split into 512-col sub-chunks so compute starts as soon as
        # the first DMA pieces land.
        chunks = [(0, 512, 0), (512, 512, 1)]
        chunks += [(k * SC, SC, 1 + k) for k in range(1, K)]
        for ci, (lo, w, ak) in enumerate(chunks):
            cs = slice(lo, lo + w)
            pb = pbu.tile([128, SC], dt, tag="bu")
            for h in range(max(1, w // 512)):
                mw = min(w, 512)
                hs = slice(h * 512, h * 512 + mw)
                for c in range(FOLD):
                    mc = c * CL + lo + h * 512
                    nc.tensor.matmul(
                        pb[c * RI : (c + 1) * RI, hs],
                        Bt[:], inpT[:, mc : mc + mw],
                        start=True, stop=True,
                        tile_position=(0, c * RI),
                    )
            Y1 = work.tile([128, SC], bt, tag="Y1")
            Y2 = work.tile([128, SC], bt, tag="Y2")
            # Both modulates on DVE: any concurrent Pool bulk op slows the
            # DVE scans ~2.4x via SBUF contention, so Pool stays idle.
            nc.vector.scalar_tensor_tensor(
                Y1[:, 0:w], pb[:, 0:w], 1.0, T1[:, cs], Alu.mult, Alu.mult)
            nc.vector.scalar_tensor_tensor(
                Y2[:, 0:w], pb[:, 0:w], 1.0, T2[:, cs], Alu.mult, Alu.mult)


            initE = zinit[:] if lo == 0 else E[:, lo - 1 : lo]
            initF = zinit[:] if lo == 0 else F[:, lo - 1 : lo]
            nc.vector.tensor_tensor_scan(
                E[:, cs], ones_b[:, 0:w], Y1[:, 0:w], initE, Alu.mult, Alu.add)
            if ci == len(chunks) - 1:
                # E totals = last scan column; copy while DVE runs scanF so
                # the Wm matmul + offs copy overlap the final scan.
                nc.scalar.activation(fins[:, 0:1], E[:, CL - 1 : CL], Ident)
            nc.vector.tensor_tensor_scan(
                F[:, cs], ones_b[:, 0:w], Y2[:, 0:w], initF, Alu.mult, Alu.add)

        # F totals from the F scan tail, then carry offsets via Wm
        nc.scalar.activation(fins[:, 1:2], F[:, CL - 1 : CL], Ident)
        po = poff.tile([128, 2], dt, tag="off")
        nc.tensor.matmul(po[:], Wm[:], fins[:], start=True, stop=True)
        nc.scalar.copy(offs[:], po[:])

        # -- phase 2: demod (bias folded into DVE stts) + project + store --
        for j in range(NJ):
            jc = j * JT
            js = slice(jc, jc + JT)
            m1 = work.tile([128, JT], bt, tag="m1")
            m2 = work.tile([128, JT], bt, tag="m2")
            x0 = work.tile([128, JT], bt, tag="x0")
            nc.vector.scalar_tensor_tensor(
                m1[:], E[:, js], offs[:, 0:1], sinT[:, js], Alu.add, Alu.mult)
            nc.vector.scalar_tensor_tensor(
                m2[:], F[:, js], offs[:, 1:2], cosT[:, js], Alu.add, Alu.mult)
            nc.vector.tensor_add(x0[:], m1[:], m2[:])
            for c in range(FOLD):
                pc = pout.tile([128, JT], dt, tag="o")
                nc.tensor.matmul(
                    pc[:], Cx[c * RI : (c + 1) * RI, :],
                    x0[c * RI : (c + 1) * RI, :],
                    start=True, stop=False,
                    tile_position=(c * RI, 0),
                )
                nc.tensor.matmul(
                    pc[:], dD[:], inpT[:, c * CL + jc : c * CL + jc + JT],
                    start=False, stop=True,
                )
                osb = opool.tile([128, JT], bt, tag="osb")
                nc.scalar.copy(osb[:], pc[:])
                # out transfers are the phase-2 bandwidth limiter: split
                # into 256-col pieces round-robined over all three queues
                for u in range(2):
                    lo = c * CL + jc + u * 256
                    oq = [nc.sync, nc.scalar, nc.gpsimd][(2 * j + 2 * c + u) % 3]
                    oq.dma_start(out=outp[:, lo : lo + 256],
                                 in_=osb[:, u * 256 : (u + 1) * 256])

        for p in (poff, pout, pbu, opool, work, big, cpool):
            p.release()
    if split_waits:
        _split_matmul_waits(nc, mybir)
    return nc


def _split_matmul_waits(nc, mybir):
    """Hardware instruction structs fit a limited number of embedded sync
    waits; move extra waits onto an inserted same-queue no-op."""
    caps = {"InstMatmult": 1}
    default_cap = 1
    skip = {"InstNoOp", "InstAllEngineBarrier", "InstSync"}
    k = 0
    for bb in nc.main_func.blocks:
        insts = bb.instructions
        i = 0
        while i < len(insts):
            ins = insts[i]
            tn = type(ins).__name__
            if tn not in skip and ins.sync_info is not None:
                cap = caps.get(tn, default_cap)
                w = list(ins.sync_info.on_wait or [])
                if len(w) > cap:
                    for wj in w[:-cap]:
                        nop = mybir.InstNoOp(
                            name=f"I-mmdep-{k}",
                            engine=ins.engine,
                            ins=[],
                            outs=[],
                            sync_info=mybir.SyncInfo(
                                on_wait=[wj], on_update=[]
                            ),
                        )
                        k += 1
                        insts.insert(i, nop)
                        i += 1
                    ins.sync_info = mybir.SyncInfo(
                        on_wait=w[-cap:], on_update=ins.sync_info.on_update
                    )
            i += 1


def _host_prep(inputs):
    import ml_dtypes
    f32 = np.float32
    bf16 = ml_dtypes.bfloat16

    inpT = np.ascontiguousarray(
        np.asarray(inputs["input_sequence"], np.float32).T
    ).astype(bf16)
    A = np.maximum(np.asarray(inputs["A_diag_raw"], np.float64), 0.0)
    s = 1.0 / (1.0 + np.exp(-np.asarray(inputs["steps_raw"], np.float64)))
    Br = np.asarray(inputs["B_real"], np.float64)
    Bi = np.asarray(inputs["B_img"], np.float64)
    Cr = np.asarray(inputs["C_real"], np.float64)
    Ci = np.asarray(inputs["C_img"], np.float64)
    D = np.asarray(inputs["D"], np.float64)

    costh = 1.0 - s * s * A / 2.0
    sinth = np.sqrt(np.maximum(1.0 - costh * costh, 1e-300))
    theta = np.arctan2(sinth, costh)
    gamma = (s - s * s * A / 2.0) / sinth

    twopi = 2.0 * np.pi
    j = np.arange(CL, dtype=np.float64)
    in_maps = []
    for kcore in range(NCORES):
        sl = slice(kcore * SLOC, (kcore + 1) * SLOC)
        th_m = np.concatenate([theta[sl], theta[sl]])       # (RI,)
        g_m = np.concatenate([gamma[sl], gamma[sl]])        # (RI,)
        # partitions p = c*RI + m,  absolute time = c*CL + j
        ang = np.empty((128, CL), np.float64)
        for c in range(FOLD):
            tt = (c * CL + j)[None, :] * th_m[:, None]
            ang[c * RI : (c + 1) * RI] = np.mod(tt, twopi)
        sinT = np.sin(ang)
        cosT = np.cos(ang)
        g2 = np.tile(g_m, FOLD)[:, None]
        T1 = g2 * cosT + sinT
        T2 = cosT - g2 * sinT

        Bt = np.empty((H, RI), np.float64)
        Bt[:, 0:SLOC] = (s[sl, None] * Br[sl]).T
        Bt[:, SLOC:RI] = (s[sl, None] * Bi[sl]).T

        Cblk = np.concatenate([Cr[:, sl].T, -Ci[:, sl].T], axis=0)  # (RI, H)
        Cx = np.tile(Cblk, (FOLD, 1))                               # (128, H)

        dD = np.diag(D) if kcore == 0 else np.zeros((H, H))
        Wm = np.zeros((128, 128), f32)
        Wm[np.arange(RI), np.arange(RI) + RI] = 1.0

        in_maps.append({
            "inpT": inpT,
            "Bt": Bt.astype(bf16),
            "Cx": Cx.astype(bf16),
            "dD": dD.astype(bf16),
            "Wm": Wm,
            "T1": T1.astype(bf16),
            "T2": T2.astype(bf16),
            "sinT": sinT.astype(bf16),
            "cosT": cosT.astype(bf16),
        })
    return in_maps


LAST_RESULTS = None


def kernel(**inputs) -> np.ndarray:
    global LAST_RESULTS
    from concourse.bass_utils import run_bass_kernel_spmd

    if "nc" not in _CACHE:
        _CACHE["nc"] = _build_bass()
    nc = _CACHE["nc"]

    in_maps = _host_prep(inputs)
    res = run_bass_kernel_spmd(nc, in_maps, core_ids=list(range(NCORES)))
    LAST_RESULTS = res
    part = np.zeros((H, L), np.float32)
    for r in res.results:
        part += np.asarray(r["outp"], np.float32)
    return np.ascontiguousarray(part.T)


# revision 87
# speedup vs baseline: 1.0024x; 1.0024x over previous
"""LinOSS layer Trainium2 kernel (pipelined, merged r/i layout).

Math (same derivation as before): per-state recurrence collapses to
rank-2 modulated prefix sums
    u_t = s * Bu_t
    E = cumsum(T1 * u);  F = cumsum(T2 * u)
    x_t = sin(t th) * (E_t + oE) + cos(t th) * (F_t + oF)
    T1 = gamma*cos + sin;  T2 = cos - gamma*sin
with oE/oF the fold-chunk carry offsets.

Layout: 128 partitions = 2 fold-chunks x (32 real + 32 imag states);
free dim = 4096 time cols.  r/i share theta so one table row set serves
both; E/F merge the four scans of the old layout into two.

Pipeline: host sends input pre-transposed plus all four tables (no
on-chip table build, no DMA transpose).  Per 1024-col chunk: Bu matmuls
-> PSUM, DVE stt modulate straight from PSUM (accum_out collects row
sums for the carry), chained DVE scans (AP initial).  Carry offsets are
ready after the last modulate, so demod (ACT bias-add, DVE/Pool mults)
and projection matmuls overlap the remaining scans.
"""

import numpy as np

L, H, P = 8192, 128, 256
NCORES = 8
SLOC = P // NCORES          # 32 states per core
RI = 2 * SLOC               # 64 r+i rows per fold chunk
FOLD = 2
CL = L // FOLD              # 4096 free cols
SC = 1024                   # modulate/scan chunk
K = CL // SC                # 4
JT = 512                    # demod/project chunk
NJ = CL // JT               # 8

_CACHE: dict = {}


def _build_bass(split_waits=True):
    import concourse.bass as bass
    import concourse.mybir as mybir
    import concourse.tile as tile

    dt = mybir.dt.float32
    bt = mybir.dt.bfloat16
    Alu = mybir.AluOpType
    Ident = mybir.ActivationFunctionType.Identity

    nc = bass.Bass(
        trn_type="TRN2",
        target_bir_lowering=False,
        debug=False,
        num_devices=NCORES,
    )

    inpT_d = nc.dram_tensor("inpT", [H, L], bt, kind="ExternalInput").ap()
    Bt_d = nc.dram_tensor("Bt", [H, RI], bt, kind="ExternalInput").ap()
    Cx_d = nc.dram_tensor("Cx", [128, H], bt, kind="ExternalInput").ap()
    dD_d = nc.dram_tensor("dD", [H, H], bt, kind="ExternalInput").ap()
    Wm_d = nc.dram_tensor("Wm", [128, 128], dt, kind="ExternalInput").ap()
    T1_d = nc.dram_tensor("T1", [128, CL], bt, kind="ExternalInput").ap()
    T2_d = nc.dram_tensor("T2", [128, CL], bt, kind="ExternalInput").ap()
    sinT_d = nc.dram_tensor("sinT", [128, CL], bt, kind="ExternalInput").ap()
    cosT_d = nc.dram_tensor("cosT", [128, CL], bt, kind="ExternalInput").ap()
    outp = nc.dram_tensor("outp", [H, L], bt, kind="ExternalOutput").ap()

    with tile.TileContext(nc) as tc:
        cpool = tc.alloc_tile_pool(name="const", bufs=1)
        big = tc.alloc_tile_pool(name="big", bufs=1)
        work = tc.alloc_tile_pool(name="work", bufs=3)
        opool = tc.alloc_tile_pool(name="opool", bufs=8)
        pbu = tc.alloc_tile_pool(name="pbu", bufs=2, space="PSUM")
        pout = tc.alloc_tile_pool(name="pout", bufs=3, space="PSUM")
        poff = tc.alloc_tile_pool(name="poff", bufs=1, space="PSUM")

        Bt = cpool.tile([H, RI], bt)
        Cx = cpool.tile([128, H], bt)
        dD = cpool.tile([H, H], bt)
        Wm = cpool.tile([128, 128], dt)
        inpT = big.tile([H, L], bt, tag="inpT")
        T1 = big.tile([128, CL], bt, tag="T1")
        T2 = big.tile([128, CL], bt, tag="T2")
        sinT = big.tile([128, CL], bt, tag="sinT")
        cosT = big.tile([128, CL], bt, tag="cosT")
        E = big.tile([128, CL], bt, tag="E")
        F = big.tile([128, CL], bt, tag="F")

        NACC = K + 1            # k=0 runs as 2 sub-chunks
        ones_b = cpool.tile([128, SC], bt)
        zinit = cpool.tile([128, 1], dt)
        acc1 = cpool.tile([128, NACC], dt)
        acc2 = cpool.tile([128, NACC], dt)
        fins = cpool.tile([128, 2], dt)
        offs = cpool.tile([128, 2], dt)

        nc.gpsimd.memset(zinit[:], 0.0)
        nc.gpsimd.memset(ones_b[:], 1.0)

        # warm the lazy ACT function table before any scalar-queue DMA
        # issues so the first real ACT op isn't delayed by the 1.3us load
        wdum = cpool.tile([128, 1], dt)
        nc.scalar.activation(wdum[:], zinit[:], Ident)

        # -- input / table DMAs --
        # One DMA's descriptors drain through a single HW queue (~25 GB/s =
        # 5.1us per 512-col piece), and each issue costs ~0.6us of its
        # sequencer, which also dispatches that engine's compute ops.  So:
        # budget few early issues per queue, strict need order, and issue
        # the rest from inside the phase-1 loop.
        def pc512(q, dst, src, lo):
            q.dma_start(out=dst[:, lo : lo + 512], in_=src[:, lo : lo + 512])

        nc.sync.dma_start(out=Bt[:], in_=Bt_d)
        pc512(nc.sync, inpT, inpT_d, 0)
        pc512(nc.scalar, inpT, inpT_d, CL)
        pc512(nc.sync, T1, T1_d, 0)
        pc512(nc.scalar, T2, T2_d, 0)
        pc512(nc.sync, inpT, inpT_d, 512)
        pc512(nc.scalar, inpT, inpT_d, CL + 512)
        pc512(nc.sync, T1, T1_d, 512)
        pc512(nc.scalar, T2, T2_d, 512)
        # k=1 pieces behind the k=0 criticals, balanced sync/scalar
        pc512(nc.scalar, inpT, inpT_d, SC)
        pc512(nc.sync, inpT, inpT_d, CL + SC)
        pc512(nc.scalar, T1, T1_d, SC)
        pc512(nc.sync, T2, T2_d, SC)
        pc512(nc.scalar, inpT, inpT_d, SC + 512)
        pc512(nc.sync, inpT, inpT_d, CL + SC + 512)
        pc512(nc.scalar, T1, T1_d, SC + 512)
        pc512(nc.sync, T2, T2_d, SC + 512)
        # k=2/3 upfront on gpsimd (~2 parallel SWDGE streams; 15-25us of
        # slack before use keeps the cold-run window closed)
        for k in range(2, K):
            for s in range(2):
                lo = k * SC + s * 512
                for c in range(FOLD):
                    pc512(nc.gpsimd, inpT, inpT_d, c * CL + lo)
                pc512(nc.gpsimd, T1, T1_d, lo)
                pc512(nc.gpsimd, T2, T2_d, lo)
        nc.sync.dma_start(out=Wm[:], in_=Wm_d)
        nc.sync.dma_start(out=Cx[:], in_=Cx_d)
        nc.sync.dma_start(out=dD[:], in_=dD_d)
        # sin/cos (phase 2 only) trail as whole chunks on the queue tails
        for k in range(K):
            cs = slice(k * SC, (k + 1) * SC)
            nc.sync.dma_start(out=sinT[:, cs], in_=sinT_d[:, cs])
            nc.scalar.dma_start(out=cosT[:, cs], in_=cosT_d[:, cs])

        # -- phase 1: Bu matmuls -> modulate -> chained scans --
        # k=0 split into 512-col sub-chunks so compute starts as soon as
        # the first DMA pieces land.
        chunks = [(0, 512, 0), (512, 512, 1)]
        chunks += [(k * SC, SC, 1 + k) for k in range(1, K)]
        for ci, (lo, w, ak) in enumerate(chunks):
            cs = slice(lo, lo + w)
            pb = pbu.tile([128, SC], dt, tag="bu")
            for h in range(max(1, w // 512)):
                mw = min(w, 512)
                hs = slice(h * 512, h * 512 + mw)
                for c in range(FOLD):
                    mc = c * CL + lo + h * 512
                    nc.tensor.matmul(
                        pb[c * RI : (c + 1) * RI, hs],
                        Bt[:], inpT[:, mc : mc + mw],
                        start=True, stop=True,
                        tile_position=(0, c * RI),
                    )
            Y1 = work.tile([128, SC], bt, tag="Y1")
            Y2 = work.tile([128, SC], bt, tag="Y2")
            # Both modulates on DVE: any concurrent Pool bulk op slows the
            # DVE scans ~2.4x via SBUF contention, so Pool stays idle.
            nc.vector.scalar_tensor_tensor(
                Y1[:, 0:w], pb[:, 0:w], 1.0, T1[:, cs], Alu.mult, Alu.mult)
            nc.vector.scalar_tensor_tensor(
                Y2[:, 0:w], pb[:, 0:w], 1.0, T2[:, cs], Alu.mult, Alu.mult)


            initE = zinit[:] if lo == 0 else E[:, lo - 1 : lo]
            initF = zinit[:] if lo == 0 else F[:, lo - 1 : lo]
            nc.vector.tensor_tensor_scan(
                E[:, cs], ones_b[:, 0:w], Y1[:, 0:w], initE, Alu.mult, Alu.add)
            if ci == len(chunks) - 1:
                # E totals = last scan column; copy while DVE runs scanF so
                # the Wm matmul + offs copy overlap the final scan.
                nc.scalar.activation(fins[:, 0:1], E[:, CL - 1 : CL], Ident)
            nc.vector.tensor_tensor_scan(
                F[:, cs], ones_b[:, 0:w], Y2[:, 0:w], initF, Alu.mult, Alu.add)

        # F totals from the F scan tail, then carry offsets via Wm
        nc.scalar.activation(fins[:, 1:2], F[:, CL - 1 : CL], Ident)
        po = poff.tile([128, 2], dt, tag="off")
        nc.tensor.matmul(po[:], Wm[:], fins[:], start=True, stop=True)
        nc.scalar.copy(offs[:], po[:])

        # -- phase 2: demod (bias folded into DVE stts) + project + store --
        for j in range(NJ):
            jc = j * JT
            js = slice(jc, jc + JT)
            m1 = work.tile([128, JT], bt, tag="m1")
            m2 = work.tile([128, JT], bt, tag="m2")
            x0 = work.tile([128, JT], bt, tag="x0")
            nc.vector.scalar_tensor_tensor(
                m1[:], E[:, js], offs[:, 0:1], sinT[:, js], Alu.add, Alu.mult)
            nc.vector.scalar_tensor_tensor(
                m2[:], F[:, js], offs[:, 1:2], cosT[:, js], Alu.add, Alu.mult)
            nc.vector.tensor_add(x0[:], m1[:], m2[:])
            for c in range(FOLD):
                pc = pout.tile([128, JT], dt, tag="o")
                nc.tensor.matmul(
                    pc[:], Cx[c * RI : (c + 1) * RI, :],
                    x0[c * RI : (c + 1) * RI, :],
                    start=True, stop=False,
                    tile_position=(c * RI, 0),
                )
                nc.tensor.matmul(
                    pc[:], dD[:], inpT[:, c * CL + jc : c * CL + jc + JT],
                    start=False, stop=True,
                )
                osb = opool.tile([128, JT], bt, tag="osb")
                nc.scalar.copy(osb[:], pc[:])
                # out transfers are the phase-2 bandwidth limiter: split
                # into 256-col pieces round-robined over all three queues
                for u in range(2):
                    lo = c * CL + jc + u * 256
                    oq = [nc.sync, nc.scalar, nc.gpsimd][(2 * j + 2 * c + u) % 3]
                    oq.dma_start(out=outp[:, lo : lo + 256],
                                 in_=osb[:, u * 256 : (u + 1) * 256])

        for p in (poff, pout, pbu, opool, work, big, cpool):
            p.release()
    if split_waits:
        _split_matmul_waits(nc, mybir)
    return nc


def _split_matmul_waits(nc, mybir):
    """Hardware instruction structs fit a limited number of embedded sync
    waits; move extra waits onto an inserted same-queue no-op."""
    caps = {"InstMatmult": 1}
    default_cap = 1
    skip = {"InstNoOp", "InstAllEngineBarrier", "InstSync"}
    k = 0
    for bb in nc.main_func.blocks:
        insts = bb.instructions
        i = 0
        while i < len(insts):
            ins = insts[i]
            tn = type(ins).__name__
            if tn not in skip and ins.sync_info is not None:
                cap = caps.get(tn, default_cap)
                w = list(ins.sync_info.on_wait or [])
                if len(w) > cap:
                    for wj in w[:-cap]:
                        nop = mybir.InstNoOp(
                            name=f"I-mmdep-{k}",
                            engine=ins.engine,
                            ins=[],
                            outs=[],
                            sync_info=mybir.SyncInfo(
                                on_wait=[wj], on_update=[]
                            ),
                        )
                        k += 1
                        insts.insert(i, nop)
                        i += 1
                    ins.sync_info = mybir.SyncInfo(
                        on_wait=w[-cap:], on_update=ins.sync_info.on_update
                    )
            i += 1


def _host_prep(inputs):
    import ml_dtypes
    f32 = np.float32
    bf16 = ml_dtypes.bfloat16

    inpT = np.ascontiguousarray(
        np.asarray(inputs["input_sequence"], np.float32).T
    ).astype(bf16)
    A = np.maximum(np.asarray(inputs["A_diag_raw"], np.float64), 0.0)
    s = 1.0 / (1.0 + np.exp(-np.asarray(inputs["steps_raw"], np.float64)))
    Br = np.asarray(inputs["B_real"], np.float64)
    Bi = np.asarray(inputs["B_img"], np.float64)
    Cr = np.asarray(inputs["C_real"], np.float64)
    Ci = np.asarray(inputs["C_img"], np.float64)
    D = np.asarray(inputs["D"], np.float64)

    costh = 1.0 - s * s * A / 2.0
    sinth = np.sqrt(np.maximum(1.0 - costh * costh, 1e-300))
    theta = np.arctan2(sinth, costh)
    gamma = (s - s * s * A / 2.0) / sinth

    twopi = 2.0 * np.pi
    j = np.arange(CL, dtype=np.float64)
    in_maps = []
    for kcore in range(NCORES):
        sl = slice(kcore * SLOC, (kcore + 1) * SLOC)
        th_m = np.concatenate([theta[sl], theta[sl]])       # (RI,)
        g_m = np.concatenate([gamma[sl], gamma[sl]])        # (RI,)
        # partitions p = c*RI + m,  absolute time = c*CL + j
        ang = np.empty((128, CL), np.float64)
        for c in range(FOLD):
            tt = (c * CL + j)[None, :] * th_m[:, None]
            ang[c * RI : (c + 1) * RI] = np.mod(tt, twopi)
        sinT = np.sin(ang)
        cosT = np.cos(ang)
        g2 = np.tile(g_m, FOLD)[:, None]
        T1 = g2 * cosT + sinT
        T2 = cosT - g2 * sinT

        Bt = np.empty((H, RI), np.float64)
        Bt[:, 0:SLOC] = (s[sl, None] * Br[sl]).T
        Bt[:, SLOC:RI] = (s[sl, None] * Bi[sl]).T

        Cblk = np.concatenate([Cr[:, sl].T, -Ci[:, sl].T], axis=0)  # (RI, H)
        Cx = np.tile(Cblk, (FOLD, 1))                               # (128, H)

        dD = np.diag(D) if kcore == 0 else np.zeros((H, H))
        Wm = np.zeros((128, 128), f32)
        Wm[np.arange(RI), np.arange(RI) + RI] = 1.0

        in_maps.append({
            "inpT": inpT,
            "Bt": Bt.astype(bf16),
            "Cx": Cx.astype(bf16),
            "dD": dD.astype(bf16),
            "Wm": Wm,
            "T1": T1.astype(bf16),
            "T2": T2.astype(bf16),
            "sinT": sinT.astype(bf16),
            "cosT": cosT.astype(bf16),
        })
    return in_maps


LAST_RESULTS = None


def kernel(**inputs) -> np.ndarray:
    global LAST_RESULTS
    from concourse.bass_utils import run_bass_kernel_spmd

    if "nc" not in _CACHE:
        _CACHE["nc"] = _build_bass()
    nc = _CACHE["nc"]

    in_maps = _host_prep(inputs)
    res = run_bass_kernel_spmd(nc, in_maps, core_ids=list(range(NCORES)))
    LAST_RESULTS = res
    part = np.zeros((H, L), np.float32)
    for r in res.results:
        part += np.asarray(r["outp"], np.float32)
    return np.ascontiguousarray(part.T)
